# revision 31
# baseline (speedup 1.0000x reference)
"""DepthSSIM loss on Trainium2 — 8-core data-parallel Bass kernel (v2).

Math (per reference):
  inp = input*mask ; tgt = target*mask
  mw  = blur(mask) + 1e-8
  mu_i = blur(inp)/mw ; mu_t = blur(tgt)/mw
  s_i2 = blur(inp^2)/mw - mu_i^2 ; s_t2 = blur(tgt^2)/mw - mu_t^2
  s_it = blur(inp*tgt)/mw - mu_i*mu_t
  L = max(max(inp), max(tgt)); C3 = (0.03 L)^2/2
  map = (s_it + C3) / (sqrt(relu(s_i2)+1e-12) sqrt(relu(s_t2)+1e-12) + C3 + 1e-8)
  loss = 1 - mean(map)

Fast path (mask identically 1 — the spec's fill for this input):
  * mw = blur(ones) is separable and data-independent; it is folded into the
    Toeplitz blur matrices (columns pre-divided by the blur-of-ones partial
    sums, then per-column bf16-sum-adjusted back to exactly 1), so the /mw
    normalization vanishes from the device program.
  * 5 blurred fields per image: i, t, i^2, t^2, i*t (bf16 data, fp32 PSUM).
  * Stage 1 (H-contraction): banded-Toeplitz matmuls with the image block
    stationary -> output lands W-major (transpose for free).
  * Stage 2 (W-contraction): Toeplitz 128x128 blocks stationary (10 (dst,src)
    block pairs), streaming the stage-1 output; epilogue consumes stage-2
    PSUM directly.
  * No collective: each core reduces with its local L_c and also emits a
    (subsampled) dLoss/dC3 partial; the host takes L = max_c L_c and applies
    a first-order correction (exact here since all cores see the same L).
A general-mask fallback (the previous collective-based program) is compiled
lazily only when mask != 1.
"""
import numpy as np
import ml_dtypes

import concourse.bass as bass
import concourse.tile as tile
from concourse import mybir
from concourse.bass_utils import run_bass_kernel_spmd
from concourse.tile import ScopedClock as _ScopedClock

# ----------------------------------------------------------------------------
# Workaround: this walrus build rejects >1 semaphore wait per instruction.
# ----------------------------------------------------------------------------
_MAX_WAITS = 1
_orig_commit = tile.TileContext._commit_instruction


def _commit_split(self, inst, lazy_reg_writes=True):
    si = getattr(inst, "sync_info", None)
    eng = getattr(inst, "engine", None)
    if si is not None and si.on_wait and len(si.on_wait) > _MAX_WAITS and eng is not None:
        waits = list(si.on_wait)
        excess, kept = waits[:-_MAX_WAITS], waits[-_MAX_WAITS:]
        for i in range(0, len(excess), _MAX_WAITS):
            nop = mybir.InstNoOp(
                name=self.nc.get_next_instruction_name(),
                engine=eng,
                sync_info=mybir.SyncInfo(on_wait=excess[i : i + _MAX_WAITS], on_update=[]),
                bass_nofuse=True,
            )
            _orig_commit(self, nop, lazy_reg_writes)
        inst.sync_info = mybir.SyncInfo(on_wait=kept, on_update=list(si.on_update or []))
    return _orig_commit(self, inst, lazy_reg_writes)


def _split_drain_and_barrier(self, tick_clock, wait_clock):
    drain_inst = self.nc.sync.drain()
    wait_clock.add_sem_waits(drain_inst.ins, _ScopedClock({None: tick_clock.global_clock}))
    si = drain_inst.ins.sync_info
    waits = list(si.on_wait) if (si is not None and si.on_wait) else []
    if len(waits) > _MAX_WAITS:
        drain_inst.ins.sync_info = mybir.SyncInfo(
            on_wait=waits[:_MAX_WAITS], on_update=list(si.on_update or [])
        )
        rest = waits[_MAX_WAITS:]
        for i in range(0, len(rest), _MAX_WAITS):
            d2 = self.nc.sync.drain()
            d2.ins.sync_info = mybir.SyncInfo(on_wait=rest[i : i + _MAX_WAITS], on_update=[])
    self.nc.all_engine_barrier()
    assert self.sems is not None
    popped = self.nc._tile_sem_poison_stack.pop()
    assert popped is self._sem_poison
    self.nc.clear_and_free_semaphores(list(self.sems.allocated().values()))
    self.nc.all_engine_barrier()


_PATCHED = False


def _apply_tile_patches():
    global _PATCHED
    if not _PATCHED:
        tile.TileContext._commit_instruction = _commit_split
        tile.TileContext._drain_and_barrier = _split_drain_and_barrier
        _PATCHED = True


# ---------------------------------------------------------------------------
# Problem constants (hardcoded per spec)
# ---------------------------------------------------------------------------
N_CORES = 8
B, H, W = 32, 512, 512
BPC = B // N_CORES          # images per core
KS, PAD = 11, 5
K2 = 0.03
HC = H // 128               # h chunks
WC = W // 128               # w chunks
BAND = 138                  # uniform stage-1 rhs width
ST = [0, 123, 251, 374]     # stage-1 out-column start per h-chunk
NF = 5                      # blurred fields: i, t, ii, tt, it
# stage-2 (dst block j, src block i) pairs with nonzero Toeplitz support
PAIRS = [(0, 0), (0, 1), (1, 0), (1, 1), (1, 2), (2, 1), (2, 2), (2, 3), (3, 2), (3, 3)]
S2_SRC = {0: [0, 1], 1: [0, 1, 2], 2: [1, 2, 3], 3: [2, 3]}

F32 = mybir.dt.float32
BF16 = mybir.dt.bfloat16
AF = mybir.ActivationFunctionType
OP = mybir.AluOpType

_CACHED_NC = None
_CACHED_NC_GENERAL = None


def _toeplitz(g):
    """T[m, j] = g[m - j + PAD]: blurred[j] = sum_m x[m] T[m, j] (zero pad)."""
    T = np.zeros((H, H), np.float64)
    idx = np.arange(H)
    for k in range(KS):
        off = k - PAD
        m = idx + off
        valid = (m >= 0) & (m < H)
        T[m[valid], idx[valid]] = g[k]
    return T


def _rank1_factors(window):
    w2d = np.asarray(window, np.float64).reshape(KS, KS)
    u, s, vt = np.linalg.svd(w2d)
    gv = u[:, 0] * np.sqrt(s[0])
    gh = vt[0, :] * np.sqrt(s[0])
    if gv.sum() < 0:
        gv, gh = -gv, -gh
    return gv, gh


def _bf16(a):
    return np.asarray(a, np.float32).astype(ml_dtypes.bfloat16)


# ---------------------------------------------------------------------------
# Fast path (mask == 1): normalized Toeplitz factors
# ---------------------------------------------------------------------------
def _bf16_col_adjust(T):
    """bf16-quantize each column, nudging taps by whole bf16 ulps so the
    fp64 sum of the quantized taps per column is exactly 1 (keeps
    blur(ones) == 1, replacing the reference's /mw normalization)."""
    Tb = T.astype(ml_dtypes.bfloat16).astype(np.float64)
    for n in range(T.shape[1]):
        col = Tb[:, n]
        nz = np.nonzero(T[:, n])[0]
        for _ in range(60):
            r = 1.0 - col[nz].sum()
            if abs(r) < 1e-7:
                break
            ulps = np.spacing(np.abs(col[nz]).astype(np.float32)).astype(np.float64) * 2 ** (23 - 7)
            cand = np.nonzero(ulps <= 2 * abs(r))[0]
            if len(cand) == 0:
                break
            k = cand[np.argmax(ulps[cand])]
            col[nz[k]] = float(
                np.asarray(np.float32(col[nz[k]] + np.sign(r) * ulps[k]), np.float32
                           ).astype(ml_dtypes.bfloat16))
        Tb[:, n] = col
    return Tb


def _norm_blocks(window):
    gv, gh = _rank1_factors(window)
    Tv = _toeplitz(gv)
    Tw = _toeplitz(gh)
    Tvn = _bf16_col_adjust(Tv / Tv.sum(axis=0)[None, :])
    Twn = _bf16_col_adjust(Tw / Tw.sum(axis=0)[None, :])
    g1 = np.zeros((HC, 128, BAND), np.float64)
    for hc in range(HC):
        g1[hc] = Tvn[128 * hc: 128 * hc + 128, ST[hc]: ST[hc] + BAND]
    g2 = np.zeros((len(PAIRS), 128, 128), np.float64)
    for p, (j, i) in enumerate(PAIRS):
        g2[p] = Twn[128 * i: 128 * (i + 1), 128 * j: 128 * (j + 1)]
    return _bf16(g1), _bf16(g2)


def _build_program_fast():
    nc = bass.Bass()

    inp_d = nc.dram_tensor("inp", [BPC, H, W], BF16, kind="ExternalInput")
    tgt_d = nc.dram_tensor("tgt", [BPC, H, W], BF16, kind="ExternalInput")
    g1_d = nc.dram_tensor("g1", [HC, 128, BAND], BF16, kind="ExternalInput")
    g2_d = nc.dram_tensor("g2", [len(PAIRS), 128, 128], BF16, kind="ExternalInput")
    psum_out_d = nc.dram_tensor("psum_out", [128, 1], F32, kind="ExternalOutput")
    dsum_out_d = nc.dram_tensor("dsum_out", [128, 1], F32, kind="ExternalOutput")
    dsum2_out_d = nc.dram_tensor("dsum2_out", [128, 1], F32, kind="ExternalOutput")
    lmax_d = nc.dram_tensor("lmax", [1, 1], F32, kind="ExternalOutput")

    SHP = [128, HC, W]          # [h%128, h//128, w] (stage-1 stationary layout)

    with tile.TileContext(nc) as tc:
        with tc.tile_pool(name="consts", bufs=1) as consts, \
             tc.tile_pool(name="imgs", bufs=1) as imgs, \
             tc.tile_pool(name="prods", bufs=2) as prods, \
             tc.tile_pool(name="vtp", bufs=2) as vtp, \
             tc.tile_pool(name="epi1", bufs=2) as epi1, \
             tc.tile_pool(name="epi2", bufs=2) as epi2, \
             tc.tile_pool(name="scr", bufs=2) as scr, \
             tc.tile_pool(name="acc", bufs=1) as acc, \
             tc.tile_pool(name="psA", bufs=3, space="PSUM") as psA, \
             tc.tile_pool(name="psB", bufs=1, space="PSUM") as psB, \
             tc.tile_pool(name="dram", bufs=1, space="DRAM") as dram:

            # ---- constants
            g1sb = consts.tile([128, HC, BAND], BF16)
            nc.sync.dma_start(out=g1sb[:], in_=g1_d.rearrange("c p b -> p c b"))
            g2sb = consts.tile([128, len(PAIRS), 128], BF16)
            nc.sync.dma_start(out=g2sb[:], in_=g2_d.rearrange("c p b -> p c b"))
            eps8 = consts.tile([128, 1], F32)
            nc.vector.memset(eps8[:], 1e-8)

            # ---- phase 0: loads + local max
            ibs, tbs = [], []
            for b in range(BPC):
                ib = imgs.tile(SHP, BF16, tag=f"ib{b}")
                nc.sync.dma_start(out=ib[:], in_=inp_d[b].rearrange("(c p) w -> p c w", p=128))
                tb = imgs.tile(SHP, BF16, tag=f"tb{b}")
                nc.sync.dma_start(out=tb[:], in_=tgt_d[b].rearrange("(c p) w -> p c w", p=128))
                ibs.append(ib)
                tbs.append(tb)

            Lcols = acc.tile([128, 2 * BPC], F32)
            for b in range(BPC):
                for k, t in enumerate((ibs[b], tbs[b])):
                    mj = scr.tile(SHP, BF16, tag="maxjunk")
                    nc.vector.tensor_scalar(out=mj[:], in0=t[:], scalar1=1.0,
                                            scalar2=-1e30, op0=OP.mult, op1=OP.max,
                                            accum_out=Lcols[:, 2 * b + k: 2 * b + k + 1])

            Lloc = acc.tile([128, 1], F32)
            nc.vector.tensor_reduce(Lloc[:], Lcols[:], axis=mybir.AxisListType.X, op=OP.max)
            lb_d = dram.tile([128, 1], F32)
            nc.sync.dma_start(out=lb_d[:], in_=Lloc[:])
            Lrow = acc.tile([1, 128], F32)
            nc.sync.dma_start(out=Lrow[:], in_=lb_d[:].rearrange("p one -> (one) (p)"))
            L11 = acc.tile([1, 1], F32)
            nc.vector.reduce_max(L11[:], Lrow[:], axis=mybir.AxisListType.X)
            l11_d = dram.tile([1, 1], F32)
            nc.sync.dma_start(out=l11_d[:], in_=L11[:])
            nc.sync.dma_start(out=lmax_d[:], in_=L11[:])
            Lbc = acc.tile([128, 1], F32)
            nc.sync.dma_start(out=Lbc[:], in_=l11_d[:].to_broadcast((128, 1)))
            c2col = acc.tile([128, 1], F32)
            nc.scalar.activation(c2col[:], Lbc[:], AF.Square, scale=K2)   # (K2 L)^2
            c3col = acc.tile([128, 1], F32)     # C3
            nc.vector.tensor_scalar_mul(c3col[:], c2col[:], 0.5)

            macc = acc.tile([128, BPC], F32)
            d1acc = acc.tile([128, BPC], F32)
            d2acc = acc.tile([128, BPC], F32)

            # 13 Act / 7 DVE rotation for stage-1 PSUM drains (Pool cannot
            # read PSUM on TRN2)
            S1_ENG = [0, 1, 0, 0, 1, 0, 0, 1, 0, 0, 1, 0, 1, 0, 0, 1, 0, 0, 1, 0]

            def emit_products(b):
                ib, tb = ibs[b], tbs[b]
                iib = prods.tile(SHP, BF16, tag="ii", name=f"ii{b}")
                nc.scalar.activation(iib[:], ib[:], AF.Square)
                ttb = prods.tile(SHP, BF16, tag="tt", name=f"tt{b}")
                nc.gpsimd.tensor_mul(ttb[:], tb[:], tb[:])
                itb = prods.tile(SHP, BF16, tag="it", name=f"it{b}")
                nc.gpsimd.tensor_mul(itb[:], ib[:], tb[:])
                return [ibs[b], tbs[b], iib, ttb, itb]

            def s1_groups(b, fields, VT):
                groups = []
                k = 0
                for fi, F in enumerate(fields):
                    for wc in range(WC):
                        def g(fi=fi, F=F, wc=wc, k=k):
                            A = psA.tile([128, W], F32, tag="A")
                            for hc in range(HC):
                                nc.tensor.matmul(
                                    A[:, ST[hc]: ST[hc] + BAND],
                                    F[:, hc, 128 * wc: 128 * (wc + 1)],
                                    g1sb[:, hc, :],
                                    start=(hc == 0), stop=(hc == HC - 1),
                                    skip_group_check=True)
                            if S1_ENG[k]:
                                nc.vector.tensor_copy(out=VT[:, wc, fi, :], in_=A[:])
                            else:
                                nc.scalar.copy(VT[:, wc, fi, :], A[:])
                        groups.append(g)
                        k += 1
                return groups

            def s2_groups(b, VT):
                XYN = epi2.tile([128, WC, 3, W], BF16, tag="XYN", name=f"XYN{b}")
                groups = []
                for j in range(WC):
                    def g(j=j):
                        Bs = [psB.tile([128, W], F32, tag=f"B{fi}", name=f"B{fi}_{b}_{j}")
                              for fi in range(NF)]
                        srcs = S2_SRC[j]
                        for fi in range(NF):
                            for si, i in enumerate(srcs):
                                pi = PAIRS.index((j, i))
                                nc.tensor.matmul(
                                    Bs[fi][:],
                                    g2sb[:, pi, :],
                                    VT[:, i, fi, :],
                                    start=(si == 0), stop=(si == len(srcs) - 1),
                                    skip_group_check=True)
                        # consume PSUM for this dst block: mu^2 / mu*mu
                        # directly from PSUM (only one PSUM operand per op)
                        pq3 = scr.tile([128, 3, W], BF16, tag="pq3")
                        qd = scr.tile([128, W], BF16, tag="qd")
                        nc.scalar.copy(qd[:], Bs[1][:])
                        nc.scalar.activation(pq3[:, 0, :], Bs[0][:], AF.Square)
                        nc.scalar.activation(pq3[:, 1, :], Bs[1][:], AF.Square)
                        nc.vector.tensor_mul(pq3[:, 2, :], Bs[0][:], qd[:])
                        # [X; Y; N0] = -[P2; Q2; PQ] + [R; S; V]
                        for kk in range(3):
                            nc.vector.scalar_tensor_tensor(
                                out=XYN[:, j, kk, :], in0=pq3[:, kk, :], scalar=-1.0,
                                in1=Bs[2 + kk][:], op0=OP.mult, op1=OP.add)
                    groups.append(g)
                return groups, None, XYN

            def emit_epilogue(b, XYN):
                Xv = XYN[:, :, 0, :]
                Yv = XYN[:, :, 1, :]
                N0v = XYN[:, :, 2, :]
                XY = epi1.tile([128, WC, W], BF16, tag="XY", name=f"XY{b}")
                nc.vector.tensor_mul(XY[:], Xv, Yv)
                XYr = epi1.tile([128, WC, W], BF16, tag="XYr", name=f"XYr{b}")
                nc.vector.tensor_scalar_max(XYr[:], XY[:], 0.0)
                # den ~= sqrt(relu(XY) + 1e-8); the C3+1e-8 den shift is
                # restored exactly to first order on the host via d1/d2 sums
                sd = epi2.tile([128, WC, W], BF16, tag="sd", name=f"sd{b}")
                nc.scalar.activation(sd[:], XYr[:], AF.Sqrt, bias=eps8[:])
                rec = epi2.tile([128, WC, W], BF16, tag="rec", name=f"rec{b}")
                with nc.allow_low_precision(reason="den in [1e-4, ~0.2]; bf16 1/D ample for 2e-2 gate"):
                    nc.vector.reciprocal(rec[:], sd[:])
                mj = scr.tile([128, WC, W], BF16, tag="maxjunk")
                nc.vector.scalar_tensor_tensor(out=mj[:], in0=N0v, scalar=c3col[:, 0:1],
                                               in1=rec[:], op0=OP.add, op1=OP.mult,
                                               accum_out=macc[:, b: b + 1])
                # d1 = sum(rec) (quarter sample, host x4): STT out=rec via
                # (1*rec) max rec, accum_out = sum(out)
                dj1 = scr.tile([128, 1, W], BF16, tag="dj1")
                nc.vector.scalar_tensor_tensor(out=dj1[:], in0=rec[:, 1:2, :], scalar=1.0,
                                               in1=rec[:, 1:2, :], op0=OP.mult, op1=OP.max,
                                               accum_out=d1acc[:, b: b + 1])
                # d2 = sum(map*rec) (quarter sample, host x4)
                dj2 = scr.tile([128, 1, W], BF16, tag="dj2")
                nc.vector.scalar_tensor_tensor(out=dj2[:], in0=mj[:, 1:2, :], scalar=1.0,
                                               in1=rec[:, 1:2, :], op0=OP.mult, op1=OP.mult,
                                               accum_out=d2acc[:, b: b + 1])

            # ---- software-pipelined emission: stage-1 of image b interleaves
            # with stage-2 of image b-1 on the PE, filling PSUM-consume stalls
            prev = None   # (b-1, s2 groups, XYN)
            for b in range(BPC):
                fields = emit_products(b)
                VT = vtp.tile([128, WC, NF, W], BF16, tag="vt", name=f"vt{b}")
                s1g = s1_groups(b, fields, VT)
                s2g = []
                if prev is not None:
                    s2g, _, prevXYN = prev[1], None, prev[2]
                for k, g in enumerate(s1g):
                    g()
                    if s2g and k % 5 == 4:
                        s2g[k // 5]()
                if prev is not None:
                    emit_epilogue(prev[0], prev[2])
                grp, PQsb, XYN = s2_groups(b, VT)
                prev = (b, grp, XYN)
            for g in prev[1]:
                g()
            emit_epilogue(prev[0], prev[2])

            mtot = acc.tile([128, 1], F32)
            nc.vector.tensor_reduce(mtot[:], macc[:], axis=mybir.AxisListType.X, op=OP.add)
            nc.sync.dma_start(out=psum_out_d[:], in_=mtot[:])
            dtot = acc.tile([128, 1], F32)
            nc.vector.tensor_reduce(dtot[:], d1acc[:], axis=mybir.AxisListType.X, op=OP.add)
            nc.sync.dma_start(out=dsum_out_d[:], in_=dtot[:])
            dtot2 = acc.tile([128, 1], F32)
            nc.vector.tensor_reduce(dtot2[:], d2acc[:], axis=mybir.AxisListType.X, op=OP.add)
            nc.sync.dma_start(out=dsum2_out_d[:], in_=dtot2[:])

    return nc


def _get_nc():
    global _CACHED_NC
    if _CACHED_NC is None:
        _apply_tile_patches()
        _CACHED_NC = _build_program_fast()
    return _CACHED_NC


def make_in_maps(input, target, mask, window):
    g1, g2 = _norm_blocks(window)
    inp = np.asarray(input, np.float32)[:, 0].astype(ml_dtypes.bfloat16)
    tgt = np.asarray(target, np.float32)[:, 0].astype(ml_dtypes.bfloat16)
    in_maps = []
    for c in range(N_CORES):
        sl = slice(c * BPC, (c + 1) * BPC)
        in_maps.append({
            "inp": np.ascontiguousarray(inp[sl]),
            "tgt": np.ascontiguousarray(tgt[sl]),
            "g1": g1, "g2": g2,
        })
    return in_maps


def finish(results):
    """loss = 1 - mean(map); per-core sums are corrected to the global C3:
    map_true ~= map_dev + (C3g - C3c)*rec - (C3g + 1e-8)*map_dev*rec
    (first order; exact to ~(C3*rec)^2 ~ 1e-5 relative)."""
    Ls = [float(np.asarray(results[c]["lmax"], np.float64)[0, 0]) for c in range(N_CORES)]
    Lg = max(Ls)
    C3g = (K2 * Lg) ** 2 / 2.0
    total = 0.0
    for c in range(N_CORES):
        s = float(np.asarray(results[c]["psum_out"], np.float64).sum())
        d1 = float(np.asarray(results[c]["dsum_out"], np.float64).sum()) * 4.0
        d2 = float(np.asarray(results[c]["dsum2_out"], np.float64).sum()) * 4.0
        C3c = (K2 * Ls[c]) ** 2 / 2.0
        total += s + (C3g - C3c) * d1 - (C3g + 1e-8) * d2
    return np.float32(1.0 - total / (B * H * W))


# ---------------------------------------------------------------------------
# General-mask fallback (previous collective-based program)
# ---------------------------------------------------------------------------
def _adjust_bf16_sum(g):
    target = g.sum()
    gb = g.astype(ml_dtypes.bfloat16).astype(np.float64)
    for _ in range(200):
        r = target - gb.sum()
        ulps = np.spacing(np.abs(gb).astype(np.float32)).astype(np.float64) * 2 ** (23 - 7)
        if abs(r) < ulps.min() / 2:
            break
        cand = np.where(ulps <= 2 * abs(r))[0]
        if len(cand) == 0:
            break
        k = cand[np.argmax(ulps[cand])]
        gb[k] = float(np.asarray(
            np.float32(gb[k] + np.sign(r) * ulps[k]), np.float32).astype(ml_dtypes.bfloat16))
    return gb


def _g_blocks(window):
    gv, gh = _rank1_factors(window)
    gv, gh = _adjust_bf16_sum(gv), _adjust_bf16_sum(gh)
    Tv = _toeplitz(gv)
    Tw = _toeplitz(gh)
    g1 = np.zeros((HC, 128, BAND), np.float64)
    for hc in range(HC):
        g1[hc] = Tv[128 * hc: 128 * hc + 128, ST[hc]: ST[hc] + BAND]
    g2 = np.zeros((WC, 128, BAND), np.float64)
    for m in range(WC):
        g2[m] = Tw[128 * m: 128 * m + 128, ST[m]: ST[m] + BAND]
    return _bf16(g1), _bf16(g2)


def _build_program_general():
    nc = bass.Bass()
    core_ids = list(range(N_CORES))

    inp_d = nc.dram_tensor("inp", [BPC, H, W], F32, kind="ExternalInput")
    tgt_d = nc.dram_tensor("tgt", [BPC, H, W], F32, kind="ExternalInput")
    msk_d = nc.dram_tensor("msk", [BPC, H, W], F32, kind="ExternalInput")
    g1_d = nc.dram_tensor("g1", [HC, 128, BAND], BF16, kind="ExternalInput")
    g2_d = nc.dram_tensor("g2", [WC, 128, BAND], BF16, kind="ExternalInput")
    psum_out_d = nc.dram_tensor("psum_out", [128, 1], F32, kind="ExternalOutput")
    lmax_d = nc.dram_tensor("lmax", [1, 1], F32, kind="ExternalOutput")

    SHP = [128, HC, W]

    with tile.TileContext(nc) as tc:
        with tc.tile_pool(name="consts", bufs=1) as consts, \
             tc.tile_pool(name="stage", bufs=3) as stage, \
             tc.tile_pool(name="fields", bufs=1) as fields, \
             tc.tile_pool(name="vtp", bufs=1) as vtp, \
             tc.tile_pool(name="btp", bufs=1) as btp, \
             tc.tile_pool(name="scrb", bufs=10) as scrb, \
             tc.tile_pool(name="scrf", bufs=3) as scrf, \
             tc.tile_pool(name="keep", bufs=1) as keep, \
             tc.tile_pool(name="acc", bufs=1) as acc, \
             tc.tile_pool(name="psv", bufs=1, space="PSUM") as psv, \
             tc.tile_pool(name="psb", bufs=1, space="PSUM") as psb, \
             tc.tile_pool(name="dram", bufs=1, space="DRAM") as dram:

            g1sb = consts.tile([128, HC, BAND], BF16)
            nc.sync.dma_start(out=g1sb[:], in_=g1_d.rearrange("c p b -> p c b"))
            g2sb = consts.tile([128, WC, BAND], BF16)
            nc.sync.dma_start(out=g2sb[:], in_=g2_d.rearrange("c p b -> p c b"))
            eps12 = consts.tile([128, 1], F32)
            nc.vector.memset(eps12[:], 1e-12)

            Lcols = acc.tile([128, 2 * BPC], F32)
            macc = acc.tile([128, BPC], F32)
            keep_np = []

            for b in range(BPC):
                inp_f = stage.tile(SHP, F32, tag="ld")
                nc.sync.dma_start(out=inp_f[:], in_=inp_d[b].rearrange("(c p) w -> p c w", p=128))
                tgt_f = stage.tile(SHP, F32, tag="ld")
                nc.sync.dma_start(out=tgt_f[:], in_=tgt_d[b].rearrange("(c p) w -> p c w", p=128))
                msk_f = stage.tile(SHP, F32, tag="ld")
                nc.sync.dma_start(out=msk_f[:], in_=msk_d[b].rearrange("(c p) w -> p c w", p=128))

                mb = fields.tile(SHP, BF16, tag="mb")
                nc.gpsimd.tensor_copy(out=mb[:], in_=msk_f[:])
                ib = scrb.tile(SHP, BF16, tag="esc")
                nc.gpsimd.tensor_copy(out=ib[:], in_=inp_f[:])
                tb = scrb.tile(SHP, BF16, tag="esc")
                nc.gpsimd.tensor_copy(out=tb[:], in_=tgt_f[:])

                inp = fields.tile(SHP, BF16, tag="inp")
                nc.vector.tensor_mul(inp[:], ib[:], mb[:])
                tgt = fields.tile(SHP, BF16, tag="tgt")
                nc.vector.tensor_mul(tgt[:], tb[:], mb[:])

                mscr = scrb.tile(SHP, BF16, tag="esc")
                nc.vector.tensor_scalar(out=mscr[:], in0=inp[:], scalar1=1.0, scalar2=-1e30,
                                        op0=OP.mult, op1=OP.max,
                                        accum_out=Lcols[:, 2 * b: 2 * b + 1])
                mscr2 = scrb.tile(SHP, BF16, tag="esc")
                nc.vector.tensor_scalar(out=mscr2[:], in0=tgt[:], scalar1=1.0, scalar2=-1e30,
                                        op0=OP.mult, op1=OP.max,
                                        accum_out=Lcols[:, 2 * b + 1: 2 * b + 2])

                s = scrb.tile(SHP, BF16, tag="esc")
                nc.vector.tensor_add(s[:], inp[:], tgt[:])
                ii = fields.tile(SHP, BF16, tag="ii")
                nc.scalar.activation(ii[:], inp[:], AF.Square)
                tt = fields.tile(SHP, BF16, tag="tt")
                nc.scalar.activation(tt[:], tgt[:], AF.Square)
                ss = fields.tile(SHP, BF16, tag="ss")
                nc.scalar.activation(ss[:], s[:], AF.Square)

                blur_in = [mb, inp, tgt, ii, tt, ss]

                vts = []
                for fi, f in enumerate(blur_in):
                    ps = psv.tile([128, WC * W], F32, tag="psv")
                    for wc in range(WC):
                        for hc in range(HC):
                            nc.tensor.matmul(
                                ps[:, wc * W + ST[hc]: wc * W + ST[hc] + BAND],
                                f[:, hc, 128 * wc: 128 * (wc + 1)],
                                g1sb[:, hc, :],
                                start=(hc == 0), stop=(hc == HC - 1),
                                skip_group_check=True)
                    vt = vtp.tile([128, WC, W], BF16, tag=f"vt{fi}")
                    nc.scalar.copy(vt[:], ps[:].rearrange("p (c w) -> p c w", c=WC))
                    vts.append(vt)

                bts = []
                for fi in range(6):
                    vt = vts[fi]
                    ps2 = psb.tile([128, WC * W], F32, tag="psb", name=f"ps2_{fi}_{b}")
                    for hb in range(HC):
                        for m in range(WC):
                            nc.tensor.matmul(
                                ps2[:, hb * W + ST[m]: hb * W + ST[m] + BAND],
                                vt[:, m, 128 * hb: 128 * (hb + 1)],
                                g2sb[:, m, :],
                                start=(m == 0), stop=(m == WC - 1),
                                skip_group_check=True)
                    if fi == 0:
                        bt = btp.tile([128, HC, W], F32, tag="bt0", name=f"bt0_{b}")
                        nc.scalar.activation(bt[:], ps2[:].rearrange("p (c w) -> p c w", c=HC),
                                             AF.Copy, bias=1e-8, scale=1.0)
                    else:
                        bt = btp.tile([128, HC, W], BF16, tag=f"bt{fi}", name=f"bt{fi}_{b}")
                        if fi % 2 == 0:
                            nc.scalar.copy(bt[:], ps2[:].rearrange("p (c w) -> p c w", c=HC))
                        else:
                            nc.vector.tensor_copy(out=bt[:], in_=ps2[:].rearrange("p (c w) -> p c w", c=HC))
                    bts.append(bt)

                Bm, Bi, Bt, Bii, Btt, Bss = bts
                rmw = scrf.tile(SHP, F32, tag="f32")
                nc.vector.reciprocal(rmw[:], Bm[:])

                c = scrb.tile(SHP, BF16, tag="esc")
                nc.vector.tensor_mul(c[:], Bi[:], rmw[:])
                d = scrb.tile(SHP, BF16, tag="esc")
                nc.vector.tensor_mul(d[:], Bt[:], rmw[:])
                e = scrb.tile(SHP, BF16, tag="esc")
                nc.gpsimd.tensor_mul(e[:], Bii[:], rmw[:])
                f_ = scrb.tile(SHP, BF16, tag="esc")
                nc.gpsimd.tensor_mul(f_[:], Btt[:], rmw[:])
                h_ = scrb.tile(SHP, BF16, tag="esc")
                nc.vector.tensor_mul(h_[:], Bss[:], rmw[:])

                cc = scrb.tile(SHP, BF16, tag="esc")
                nc.scalar.activation(cc[:], c[:], AF.Square)
                dd = scrb.tile(SHP, BF16, tag="esc")
                nc.scalar.activation(dd[:], d[:], AF.Square)
                si2 = scrb.tile(SHP, BF16, tag="esc")
                nc.vector.tensor_sub(si2[:], e[:], cc[:])
                st2 = scrb.tile(SHP, BF16, tag="esc")
                nc.vector.tensor_sub(st2[:], f_[:], dd[:])
                si2r = scrb.tile(SHP, BF16, tag="esc")
                nc.vector.tensor_scalar_max(si2r[:], si2[:], 0.0)
                st2r = scrb.tile(SHP, BF16, tag="esc")
                nc.vector.tensor_scalar_max(st2r[:], st2[:], 0.0)
                si = scrb.tile(SHP, BF16, tag="esc")
                nc.scalar.activation(si[:], si2r[:], AF.Sqrt, bias=eps12[:])
                st = scrb.tile(SHP, BF16, tag="esc")
                nc.scalar.activation(st[:], st2r[:], AF.Sqrt, bias=eps12[:])

                t2 = scrb.tile(SHP, BF16, tag="esc")
                nc.gpsimd.tensor_sub(t2[:], h_[:], e[:])
                t2b = scrb.tile(SHP, BF16, tag="esc")
                nc.gpsimd.tensor_sub(t2b[:], t2[:], f_[:])
                cd = scrb.tile(SHP, BF16, tag="esc")
                nc.gpsimd.tensor_mul(cd[:], c[:], d[:])

                denp = keep.tile(SHP, BF16, tag=f"denp{b}")
                nc.vector.tensor_mul(denp[:], si[:], st[:])
                num2 = keep.tile(SHP, BF16, tag=f"num2{b}")
                nc.vector.scalar_tensor_tensor(out=num2[:], in0=cd[:], scalar=-2.0,
                                               in1=t2b[:], op0=OP.mult, op1=OP.add)
                keep_np.append((num2, denp))

            Lloc = acc.tile([128, 1], F32)
            nc.vector.tensor_reduce(Lloc[:], Lcols[:], axis=mybir.AxisListType.X, op=OP.max)
            lb_d = dram.tile([128, 1], F32)
            nc.sync.dma_start(out=lb_d[:], in_=Lloc[:])
            Lrow = acc.tile([1, 128], F32)
            nc.sync.dma_start(out=Lrow[:], in_=lb_d[:].rearrange("p one -> (one) (p)"))
            L11 = acc.tile([1, 1], F32)
            nc.vector.reduce_max(L11[:], Lrow[:], axis=mybir.AxisListType.X)
            ccin = dram.tile([1, 1], F32)
            nc.sync.dma_start(out=ccin[:], in_=L11[:])
            ccout = dram.tile([1, 1], F32)
            nc.gpsimd.collective_compute(
                "AllReduce", OP.max, replica_groups=[core_ids],
                ins=[ccin[:]], outs=[ccout[:]])
            nc.sync.dma_start(out=lmax_d[:], in_=ccout[:])
            Lbc = acc.tile([128, 1], F32)
            nc.sync.dma_start(out=Lbc[:], in_=ccout[:].to_broadcast((128, 1)))
            twoC3 = acc.tile([128, 1], F32)
            nc.scalar.activation(twoC3[:], Lbc[:], AF.Square, scale=K2)
            twoC3e = acc.tile([128, 1], F32)
            nc.vector.tensor_scalar_add(twoC3e[:], twoC3[:], 2e-8)

            for b in range(BPC):
                num2, denp = keep_np[b]
                den2 = scrf.tile(SHP, F32, tag="f32")
                nc.vector.tensor_scalar(out=den2[:], in0=denp[:], scalar1=2.0,
                                        scalar2=twoC3e[:, 0:1], op0=OP.mult, op1=OP.add)
                rec = scrf.tile(SHP, F32, tag="f32")
                nc.vector.reciprocal(rec[:], den2[:])
                mout = scrb.tile(SHP, BF16, tag="esc")
                nc.vector.scalar_tensor_tensor(out=mout[:], in0=num2[:], scalar=twoC3[:, 0:1],
                                               in1=rec[:], op0=OP.add, op1=OP.mult,
                                               accum_out=macc[:, b: b + 1])

            mtot = acc.tile([128, 1], F32)
            nc.vector.tensor_reduce(mtot[:], macc[:], axis=mybir.AxisListType.X, op=OP.add)
            nc.sync.dma_start(out=psum_out_d[:], in_=mtot[:])

    return nc


def _get_nc_general():
    global _CACHED_NC_GENERAL
    if _CACHED_NC_GENERAL is None:
        _apply_tile_patches()
        _CACHED_NC_GENERAL = _build_program_general()
    return _CACHED_NC_GENERAL


def _make_in_maps_general(input, target, mask, window):
    g1, g2 = _g_blocks(window)
    inp = np.ascontiguousarray(np.asarray(input, np.float32)[:, 0])
    tgt = np.ascontiguousarray(np.asarray(target, np.float32)[:, 0])
    msk = np.ascontiguousarray(np.asarray(mask, np.float32)[:, 0])
    in_maps = []
    for c in range(N_CORES):
        sl = slice(c * BPC, (c + 1) * BPC)
        in_maps.append({
            "inp": inp[sl], "tgt": tgt[sl], "msk": msk[sl],
            "g1": g1, "g2": g2,
        })
    return in_maps


def _finish_general(results):
    total = 0.0
    for c in range(N_CORES):
        total += float(np.asarray(results[c]["psum_out"], np.float64).sum())
    return np.float32(1.0 - total / (B * H * W))


def kernel(input, target, mask, window):
    if np.all(np.asarray(mask) == 1.0):
        nc = _get_nc()
        in_maps = make_in_maps(input, target, mask, window)
        res = run_bass_kernel_spmd(nc, in_maps, list(range(N_CORES)))
        return finish(res.results)
    nc = _get_nc_general()
    in_maps = _make_in_maps_general(input, target, mask, window)
    res = run_bass_kernel_spmd(nc, in_maps, list(range(N_CORES)))
    return _finish_general(res.results)
